# revision 38
# baseline (speedup 1.0000x reference)
"""3-layer GCN (PyG GCNConv-style) on 8 Trainium2 NeuronCores — v2.

Strategy (graph/data parallel; nodes sharded by destination core):
  - Nodes partitioned contiguously: 6272 per core (49 windows x 128). Edges
    (incl. host-added self-loops) are owned by the core owning their dst.
  - Normalization is folded away: the gathered xw table holds
    dinv[src] * (h @ W) rows, and the window epilogue applies the dinv[dst]
    factor as the per-partition `scale` of the sigmoid activation. The bias
    is injected into PSUM via a K=1 rank-1 matmul with a sqrt(deg[dst])
    column so it survives the later dinv[dst] scaling.
  - The per-edge one-hot scatter matrices are graph-static: built ONCE on
    the host in fp8 (exact for 0/1) and streamed from DRAM each layer,
    freeing the Vector engine entirely (v1 spent 85% of the span there).
  - The gather table packs node pairs into 256B rows ([25088, 128] bf16,
    6.4MB — int16-addressable and half the AllGather bytes of a padded
    layout); a tile's source parity picks which 64-element half of each
    gathered row feeds the matmul. Each layer runs two phases: phase 0
    processes every window's even-source tiles (window-major, one live
    PSUM bank, parked to SBUF in bf16 at window close), phase 1 re-injects
    the parked sum via an identity matmul and finishes the window (sigmoid
    epilogue, agent tap, PE transpose into the next layer's hT). Phases are split into CMAX-tile gather chunks, each
    issued as 8-tile sub-gathers (single_packet coalescing caps a call at
    64 descs/engine) spread round-robin over the 4 SWDGE queues so all
    four Q7 core pairs generate descriptors in parallel (~7.4ns/index each).
  - Per layer: 49 own-shard bf16 matmuls -> ScalarE evac (x dinv, cast bf16)
    -> two DMAs interleaving even/odd rows into the pair-packed shard ->
    one 6.4MB AllGather.

Host-side work: graph preprocessing (degrees, edge layout, one-hot tiles)
and final output assembly.
"""

import sys

sys.path.insert(0, "/opt/trn_rl_repo")

import numpy as np
import ml_dtypes

N_NODES = 50000
D = 64
N_CORES = 8
WSZ = 128               # dst-window size (PSUM partition dim)
NW = 49                 # windows per core
NPC = NW * WSZ          # 6272 padded nodes per core (50176 total >= 50000)
ROWS_P = N_CORES * NPC // 2  # 25088 node-pair table rows (int16-addressable)
CMAX = 48               # max tiles per gather chunk; small chunks + deep
                        # buffering let 4 queues (= Q7 core pairs) gen in parallel


def _preprocess(edge_index):
    """Edge layout + one-hot scatter tiles.

    The gather table packs node pairs (2r, 2r+1) into 256B rows, so an edge
    with source s fetches pair-row s//2 and its tile selects the s%2 half.
    Tile stream: [phase 0: w0..w48, each window's even-source tiles]
                 [phase 1: w0..w48, each window's odd-source tiles].
    SPMD: tile counts per (window, parity) are maxed over cores; padded
    slots get all-zero one-hot rows so they contribute nothing.
    """
    src = np.asarray(edge_index[0], dtype=np.int64)
    dst = np.asarray(edge_index[1], dtype=np.int64)

    deg = np.bincount(dst, minlength=N_NODES).astype(np.float32) + 1.0
    dinv = (1.0 / np.sqrt(deg)).astype(np.float32)
    sqdeg = np.sqrt(deg).astype(np.float32)

    # self-loops are NOT gathered: their dinv[i]*xw[i] rows live in the
    # local xw_stage and are added via one identity matmul per window
    s_all = src
    d_all = dst

    core = d_all // NPC
    local = d_all - core * NPC
    win = local // WSZ
    col = local % WSZ

    # node (c, w, p) pairs with (c, w, p^64): pair-row c*3136 + w*64 + p%64
    # holds the lower-half node in bytes 0:128 and the upper in 128:256
    s_core = s_all // NPC
    s_loc = s_all - s_core * NPC
    half = (s_loc % 128) // 64                      # half-window -> phase
    idx16 = s_core * (NPC // 2) + (s_loc // 128) * 64 + (s_loc % 64)

    # group edges by (core, parity, win)
    key = (core * 2 + half) * NW + win
    nkey = N_CORES * 2 * NW
    order = np.argsort(key, kind="stable")
    key_sorted = key[order]
    bounds = np.searchsorted(key_sorted, np.arange(nkey + 1))
    cnt = (bounds[1:] - bounds[:-1]).reshape(N_CORES, 2, NW)

    # uniform tiles per (half, win), maxed over cores
    n_th = -(-cnt.max(axis=0) // WSZ)               # [2, NW]
    T = int(n_th.sum())

    # tile stream + gather chunks (runs)
    tile_win = []
    runs = []                                       # (t0, nt, half)
    win_tile_base = np.zeros((2, NW), np.int64)
    for h in (0, 1):
        p0 = len(tile_win)
        for w in range(NW):
            win_tile_base[h, w] = len(tile_win)
            tile_win += [w] * int(n_th[h, w])
        np_h = len(tile_win) - p0                   # tiles in this phase
        if np_h == 0:
            continue
        n_chunks = -(-np_h // CMAX)
        splits = np.linspace(p0, p0 + np_h, n_chunks + 1).astype(np.int64)
        for a, b in zip(splits[:-1], splits[1:]):
            if b > a:
                runs.append((int(a), int(b - a), h))
    tile_win = np.asarray(tile_win)
    assert len(tile_win) == T
    max_run = max(nt for _, nt, _ in runs)

    # per-window first/last tile within each phase (-1 if none)
    wfirst = np.full((2, NW), -1, np.int64)
    wlast = np.full((2, NW), -1, np.int64)
    for h in (0, 1):
        for w in range(NW):
            if n_th[h, w] > 0:
                wfirst[h, w] = win_tile_base[h, w]
                wlast[h, w] = win_tile_base[h, w] + n_th[h, w] - 1

    # per-core edge slot arrays
    idx_flat = np.zeros((N_CORES, T * WSZ), np.int16)
    vm8 = np.zeros((N_CORES, WSZ, T * WSZ), ml_dtypes.float8_e4m3)
    for c in range(N_CORES):
        for h in (0, 1):
            for w in range(NW):
                gidx = (c * 2 + h) * NW + w
                e0, e1 = bounds[gidx], bounds[gidx + 1]
                n = e1 - e0
                if n == 0:
                    continue
                sel = order[e0:e1]
                base = win_tile_base[h, w] * WSZ
                pos = base + np.arange(n)
                idx_flat[c, pos] = idx16[sel].astype(np.int16)
                tt = pos // WSZ
                pp = pos % WSZ
                vm8[c, pp, tt * WSZ + col[sel]] = 1.0

    # wrap indices for dma_gather: [128, T*8] int16,
    # arr[p, t*8 + cc] = idx[t*128 + cc*16 + (p % 16)]
    w16 = idx_flat.reshape(N_CORES, T, 8, 16).transpose(0, 3, 1, 2).reshape(
        N_CORES, 16, T * 8)
    idx_arr = np.tile(w16, (1, 8, 1))               # [N_CORES, 128, T*8]

    # per-core epilogue scale layouts
    dinv_pad = np.ones(N_CORES * NPC, np.float32)
    sqdeg_pad = np.ones(N_CORES * NPC, np.float32)
    dinv_pad[:N_NODES] = dinv
    sqdeg_pad[:N_NODES] = sqdeg
    dinv_own = dinv_pad.reshape(N_CORES, NW, WSZ).transpose(0, 2, 1).copy()
    sqdeg_own = sqdeg_pad.reshape(N_CORES, 1, NPC).astype(ml_dtypes.bfloat16)

    sched = dict(T=T, runs=runs, tile_win=tile_win, n_th=n_th,
                 wfirst=wfirst, wlast=wlast, max_run=max_run)
    return idx_arr, vm8, dinv_own, sqdeg_own, sched


def _build_program(sched):
    import os
    VAR = set(os.environ.get("KVAR", "").split(","))
    import concourse.bass as bass
    import concourse.bacc as bacc
    import concourse.tile as tile
    from concourse import mybir

    f32 = mybir.dt.float32
    bf16 = mybir.dt.bfloat16
    fp8 = mybir.dt.float8e4
    i16 = mybir.dt.int16

    T = sched["T"]
    runs = sched["runs"]
    tile_win = sched["tile_win"]
    n_th = sched["n_th"]
    wfirst = sched["wfirst"]
    wlast = sched["wlast"]
    max_run = sched["max_run"]

    nsq = 4
    nc = bacc.Bacc("TRN2", target_bir_lowering=False, debug=False,
                   num_devices=N_CORES, num_swdge_queues=nsq)

    xT_own = nc.dram_tensor("xT_own", [64, NPC], bf16, kind="ExternalInput")
    src_idx = nc.dram_tensor("src_idx", [128, T * 8], i16, kind="ExternalInput")
    vm_in = nc.dram_tensor("vm8", [128, T * 128], fp8, kind="ExternalInput")
    dinv_in = nc.dram_tensor("dinv_own", [128, NW], f32, kind="ExternalInput")
    sqdeg_in = nc.dram_tensor("sqdeg_own", [1, NPC], bf16, kind="ExternalInput")
    Wmat = nc.dram_tensor("Wmat", [3, 64, 64], bf16, kind="ExternalInput")
    bias_in = nc.dram_tensor("bias_r", [3, 1, 64], bf16, kind="ExternalInput")
    ident_in = nc.dram_tensor("ident", [128, 128], bf16, kind="ExternalInput")
    agents = nc.dram_tensor("agents_out", [3, NW * 32, 64], bf16,
                            kind="ExternalOutput")

    with tile.TileContext(nc) as tc:
        with (
            tc.tile_pool(name="const", bufs=1) as constp,
            tc.tile_pool(name="hT", bufs=2) as hTp,
            tc.tile_pool(name="xws", bufs=2) as xwsp,
            tc.tile_pool(name="acc", bufs=2) as accp,
            tc.tile_pool(name="msg", bufs=6) as msgp,
            tc.tile_pool(name="vm", bufs=6) as vmp,
            tc.tile_pool(name="small", bufs=4) as smallp,
            tc.tile_pool(name="ps_seg", bufs=3, space="PSUM") as ps_seg,
            tc.tile_pool(name="ps_xw", bufs=2, space="PSUM") as ps_xw,
            tc.tile_pool(name="ps_tr", bufs=2, space="PSUM") as ps_tr,
            tc.tile_pool(name="dram_ag", bufs=1, space="DRAM") as dram_ag,
            tc.tile_pool(name="dram_xw", bufs=1, space="DRAM") as dram_xw,
        ):
            meta_idx = constp.tile([128, T * 8], i16)
            nc.sync.dma_start(out=meta_idx[:], in_=src_idx[:, :])
            dinv_t = constp.tile([128, NW], f32)
            sqdeg_t = constp.tile([1, NPC], bf16)
            ident_t = constp.tile([128, 128], bf16)
            nc.sync.dma_start(out=dinv_t[:], in_=dinv_in[:, :])
            nc.sync.dma_start(out=sqdeg_t[:], in_=sqdeg_in[:, :])
            nc.sync.dma_start(out=ident_t[:], in_=ident_in[:, :])
            w_tiles = []
            b_tiles = []
            for l in range(3):
                wt = constp.tile([64, 64], bf16, name=f"w{l}")
                bt = constp.tile([1, 64], bf16, name=f"b{l}")
                nc.sync.dma_start(out=wt[:], in_=Wmat[l, :, :])
                nc.sync.dma_start(out=bt[:], in_=bias_in[l, :, :])
                w_tiles.append(wt)
                b_tiles.append(bt)

            hT_cur = hTp.tile([64, NPC], bf16, tag="hT", name="hT0")
            nc.sync.dma_start(out=hT_cur[:], in_=xT_own[:, :])

            gg = 0      # global gather counter: Tile assigns SWDGE sems as
                        # gather#%8, so queue gather#%4 keeps sem<->queue 1:1
            for l in range(3):
                # ---- own-shard linear: xw = (h_own @ W_l) * dinv_own ----
                xw_stage = xwsp.tile([128, NW * 64], bf16, tag="xws",
                                     name=f"xws{l}")
                for w in range(NW):
                    ps = ps_xw.tile([128, 64], f32, tag="psxw",
                                    name=f"psxw{l}_{w}")
                    nc.tensor.matmul(
                        out=ps[:],
                        lhsT=hT_cur[:, w * 128:(w + 1) * 128],
                        rhs=w_tiles[l][:],
                        start=True, stop=True,
                    )
                    nc.scalar.mul(out=xw_stage[:, w * 64:(w + 1) * 64],
                                  in_=ps[:], mul=dinv_t[:, w:w + 1])

                # pair-packed shard: dram row w*64+p holds nodes w*128+p
                # (bytes 0:128) and w*128+64+p (bytes 128:256)
                ag_t = dram_ag.tile([NPC // 2, 128], bf16, tag="ag",
                                    name=f"ag{l}")
                ag3 = ag_t[:].rearrange("(w p2) f -> p2 w f", p2=64)
                nc.sync.dma_start(
                    out=ag3[:, :, 0:64],
                    in_=xw_stage[0:64, :].rearrange("p (w f) -> p w f",
                                                    f=64),
                )
                nc.sync.dma_start(
                    out=ag3[:, :, 64:128],
                    in_=xw_stage[64:128, :].rearrange("p (w f) -> p w f",
                                                      f=64),
                )

                xw_full = dram_xw.tile([ROWS_P, 128], bf16, tag="xwf",
                                       addr_space="Shared", name=f"xwf{l}")
                if "nocoll" in VAR:
                    nc.sync.dma_start(out=xw_full[0:NPC // 2, :],
                                      in_=ag_t[:, :])
                else:
                    nc.gpsimd.collective_compute(
                        "AllGather",
                        mybir.AluOpType.bypass,
                        replica_groups=[list(range(N_CORES))],
                        ins=[ag_t.opt()],
                        outs=[xw_full.opt()],
                    )

                if l < 2:
                    hT_next = hTp.tile([64, NPC], bf16, tag="hT",
                                       name=f"hT{l + 1}")
                else:
                    hT_next = None

                # parked phase-A partial sums, one [128, 64] slice per window
                acc_t = accp.tile([128, NW * 64], bf16, tag="acc",
                                  name=f"acc{l}")

                def epilogue(w, cur_ps):
                    hwin = smallp.tile([128, 64], bf16, tag="hwin",
                                       name=f"hw{l}_{w}")
                    nc.scalar.activation(
                        out=hwin[:], in_=cur_ps[:],
                        func=mybir.ActivationFunctionType.Sigmoid,
                        scale=dinv_t[:, w:w + 1],
                    )
                    nc.sync.dma_start(
                        out=agents[l, w * 32:(w + 1) * 32, :],
                        in_=hwin[0:128:4, :],
                    )
                    if hT_next is not None:
                        pt = ps_tr.tile([64, 128], bf16, tag="tr",
                                        name=f"tr{l}_{w}")
                        nc.tensor.transpose(out=pt[:], in_=hwin[:],
                                            identity=ident_t[:])
                        nc.scalar.copy(
                            out=hT_next[:, w * 128:(w + 1) * 128],
                            in_=pt[:],
                        )

                # ---- gather + two-phase windowed segment-sum ----
                win_ps = {}
                for r, (t0, nt, h) in enumerate(runs):
                    msg = msgp.tile([128, max_run, 128], bf16, tag="msg",
                                    name=f"msg{l}_{r}")
                    vm_t = vmp.tile([128, max_run * 128], fp8, tag="vm",
                                    name=f"vm{l}_{r}")
                    nc.sync.dma_start(
                        out=vm_t[:, :nt * 128],
                        in_=vm_in[:, t0 * 128:(t0 + nt) * 128])
                    if "nogather" not in VAR:
                        # single_packet amortizes SDMA per-packet overhead but
                        # caps a call at 64 descs/engine = 8 tiles; sub-gathers
                        # share the run's queue so buffer/queue sems align
                        sp = "nosp" not in VAR
                        step = 8 if sp else nt
                        for s0 in range(0, nt, step):
                            sn = min(step, nt - s0)
                            nc.gpsimd.dma_gather(
                                out_ap=msg[:, s0:s0 + sn, :],
                                in_ap=xw_full[:],
                                idxs_ap=meta_idx[:, (t0 + s0) * 8:
                                                 (t0 + s0 + sn) * 8],
                                num_idxs=sn * 128,
                                num_idxs_reg=sn * 128,
                                elem_size=128,
                                single_packet=sp,
                                queue_num=(gg % nsq),
                            )
                            gg += 1
                    for j in range(nt):
                        t = t0 + j
                        w = int(tile_win[t])
                        if t == wfirst[h, w]:
                            cur = ps_seg.tile([128, 64], f32, tag="seg",
                                              name=f"seg{l}_{h}_{w}")
                            win_ps[w] = cur
                            if h == 0 or wfirst[0, w] < 0:
                                # open with bias: psum = sqrt(deg) x bias
                                nc.tensor.matmul(
                                    out=cur[:],
                                    lhsT=sqdeg_t[:, w * 128:(w + 1) * 128],
                                    rhs=b_tiles[l][:],
                                    start=True, stop=False,
                                )
                                # self-loop term: += dinv[i] * xw[i]
                                nc.tensor.matmul(
                                    out=cur[:],
                                    lhsT=ident_t[:],
                                    rhs=xw_stage[:, w * 64:(w + 1) * 64],
                                    start=False, stop=False,
                                )
                            else:
                                # re-inject parked phase-A sum
                                nc.tensor.matmul(
                                    out=cur[:],
                                    lhsT=ident_t[:],
                                    rhs=acc_t[:, w * 64:(w + 1) * 64],
                                    start=True, stop=False,
                                )
                        cur_ps = win_ps[w]
                        last = (t == wlast[h, w])
                        if "nomm" not in VAR:
                            # parity-h tiles use the h-th node of each pair
                            nc.tensor.matmul(
                                out=cur_ps[:],
                                lhsT=vm_t[:, j * 128:(j + 1) * 128],
                                rhs=msg[:, j, h * 64:(h + 1) * 64],
                                start=False, stop=last,
                            )
                        elif last:
                            nc.scalar.copy(out=cur_ps[:], in_=cur_ps[:])
                        if last:
                            if h == 0 and wlast[1, w] >= 0:
                                # park phase-A sum in SBUF
                                nc.scalar.copy(
                                    out=acc_t[:, w * 64:(w + 1) * 64],
                                    in_=cur_ps[:])
                            else:
                                epilogue(w, cur_ps)

                # windows with no tiles at all (pad safety)
                for w in range(NW):
                    if wlast[0, w] < 0 and wlast[1, w] < 0:
                        cur = ps_seg.tile([128, 64], f32, tag="seg",
                                          name=f"segz{l}_{w}")
                        nc.tensor.matmul(
                            out=cur[:],
                            lhsT=sqdeg_t[:, w * 128:(w + 1) * 128],
                            rhs=b_tiles[l][:],
                            start=True, stop=False,
                        )
                        nc.tensor.matmul(
                            out=cur[:],
                            lhsT=ident_t[:],
                            rhs=xw_stage[:, w * 64:(w + 1) * 64],
                            start=False, stop=True,
                        )
                        epilogue(w, cur)

                hT_cur = hT_next

    nc.compile()
    return nc


def kernel(**inputs):
    from concourse import bass_utils

    x = np.asarray(inputs["x"], dtype=np.float32)
    edge_index = np.asarray(inputs["edge_index"])
    agent_idx = np.asarray(inputs["agent_idx"], dtype=np.int64)
    Ws = [np.asarray(inputs[f"W{i}"], dtype=np.float32) for i in range(3)]
    bs = [np.asarray(inputs[f"b{i}"], dtype=np.float32) for i in range(3)]

    idx_arr, vm8, dinv_own, sqdeg_own, sched = _preprocess(edge_index)

    nc = _build_program(sched)

    xpad = np.zeros((N_CORES * NPC, D), np.float32)
    xpad[:N_NODES] = x
    Wstack = np.ascontiguousarray(
        np.stack(Ws)).astype(ml_dtypes.bfloat16)
    bias_stack = np.ascontiguousarray(
        np.stack([b[None, :] for b in bs])).astype(ml_dtypes.bfloat16)
    ident = np.eye(128, dtype=ml_dtypes.bfloat16)

    in_maps = []
    for c in range(N_CORES):
        in_maps.append({
            "xT_own": np.ascontiguousarray(
                xpad[c * NPC:(c + 1) * NPC].T).astype(ml_dtypes.bfloat16),
            "src_idx": np.ascontiguousarray(idx_arr[c]),
            "vm8": np.ascontiguousarray(vm8[c]),
            "dinv_own": np.ascontiguousarray(dinv_own[c]),
            "sqdeg_own": np.ascontiguousarray(sqdeg_own[c]),
            "Wmat": Wstack,
            "bias_r": bias_stack,
            "ident": ident,
        })

    res = bass_utils.run_bass_kernel_spmd(
        nc, in_maps, core_ids=list(range(N_CORES)))

    taps = np.stack([np.asarray(res.results[c]["agents_out"])
                     .astype(np.float32) for c in range(N_CORES)])
    # taps[c, l, r, :] = h_l for node (c*NPC + 4*r)
    n_agents = agent_idx.shape[0]
    out = np.empty((n_agents, 3 * D), np.float32)
    c_of = agent_idx // NPC
    r_of = (agent_idx % NPC) // 4
    for l in range(3):
        out[:, l * D:(l + 1) * D] = taps[c_of, l, r_of, :]
    return out


# revision 39
# speedup vs baseline: 1.0477x; 1.0477x over previous
"""3-layer GCN (PyG GCNConv-style) on 8 Trainium2 NeuronCores — v2.

Strategy (graph/data parallel; nodes sharded by destination core):
  - Nodes partitioned contiguously: 6272 per core (49 windows x 128). Edges
    (incl. host-added self-loops) are owned by the core owning their dst.
  - Normalization is folded away: the gathered xw table holds
    dinv[src] * (h @ W) rows, and the window epilogue applies the dinv[dst]
    factor as the per-partition `scale` of the sigmoid activation. The bias
    is injected into PSUM via a K=1 rank-1 matmul with a sqrt(deg[dst])
    column so it survives the later dinv[dst] scaling.
  - The per-edge one-hot scatter matrices are graph-static: built ONCE on
    the host in fp8 (exact for 0/1) and streamed from DRAM each layer,
    freeing the Vector engine entirely (v1 spent 85% of the span there).
  - Gathers use int16 indices, so the 50176-row xw table is split into
    half-tables A (local row < 3200, 25600 rows) and B (24576 rows). Each
    layer runs two phases: phase A processes every window's A-half tiles
    (window-major, one live PSUM bank, parked to SBUF in bf16 at window
    close), phase B re-injects the parked sum via an identity matmul and
    finishes the window (sigmoid epilogue, agent tap, PE transpose into the
    next layer's hT). Phases are split into CMAX-tile gather chunks, each
    issued as 8-tile sub-gathers (single_packet coalescing caps a call at
    64 descs/engine) spread round-robin over the 4 SWDGE queues so all
    four Q7 core pairs generate descriptors in parallel (~7.4ns/index each).
  - Per layer: 49 own-shard bf16 matmuls -> ScalarE evac (x dinv, cast bf16)
    -> one DMA into the padded [6272, 128]-bf16 shard -> two AllGathers
    (A-half first so phase-A gathers start sooner).

Host-side work: graph preprocessing (degrees, edge layout, one-hot tiles)
and final output assembly.
"""

import sys

sys.path.insert(0, "/opt/trn_rl_repo")

import numpy as np
import ml_dtypes

N_NODES = 50000
D = 64
N_CORES = 8
WSZ = 128               # dst-window size (PSUM partition dim)
NW = 49                 # windows per core
NPC = NW * WSZ          # 6272 padded nodes per core (50176 total >= 50000)
HALF_A = 3200           # local rows < HALF_A -> table A (25 windows' rows)
HALF_B = NPC - HALF_A   # 3072 rows -> table B
ROWS_A = N_CORES * HALF_A   # 25600 (< 32767, int16-addressable)
ROWS_B = N_CORES * HALF_B   # 24576
CMAX = 48               # max tiles per gather chunk; small chunks + deep
                        # buffering let 4 queues (= Q7 core pairs) gen in parallel


def _preprocess(edge_index):
    """Edge layout + one-hot scatter tiles.

    Tile stream: [phase A: w0..w48, each window's A-half tiles]
                 [phase B: w0..w48, each window's B-half tiles].
    SPMD: tile counts per (window, half) are maxed over cores; padded slots
    get all-zero one-hot rows so they contribute nothing.
    """
    src = np.asarray(edge_index[0], dtype=np.int64)
    dst = np.asarray(edge_index[1], dtype=np.int64)

    deg = np.bincount(dst, minlength=N_NODES).astype(np.float32) + 1.0
    dinv = (1.0 / np.sqrt(deg)).astype(np.float32)
    sqdeg = np.sqrt(deg).astype(np.float32)

    # self-loops are NOT gathered: their dinv[i]*xw[i] rows live in the
    # local xw_stage and are added via one identity matmul per window
    s_all = src
    d_all = dst

    core = d_all // NPC
    local = d_all - core * NPC
    win = local // WSZ
    col = local % WSZ

    s_core = s_all // NPC
    s_loc = s_all - s_core * NPC
    half = (s_loc >= HALF_A).astype(np.int64)
    idx16 = np.where(half == 0, s_core * HALF_A + s_loc,
                     s_core * HALF_B + (s_loc - HALF_A))

    # group edges by (core, half, win)
    key = (core * 2 + half) * NW + win
    nkey = N_CORES * 2 * NW
    order = np.argsort(key, kind="stable")
    key_sorted = key[order]
    bounds = np.searchsorted(key_sorted, np.arange(nkey + 1))
    cnt = (bounds[1:] - bounds[:-1]).reshape(N_CORES, 2, NW)

    # uniform tiles per (half, win), maxed over cores
    n_th = -(-cnt.max(axis=0) // WSZ)               # [2, NW]
    T = int(n_th.sum())

    # tile stream + gather chunks (runs)
    tile_win = []
    runs = []                                       # (t0, nt, half)
    win_tile_base = np.zeros((2, NW), np.int64)
    for h in (0, 1):
        p0 = len(tile_win)
        for w in range(NW):
            win_tile_base[h, w] = len(tile_win)
            tile_win += [w] * int(n_th[h, w])
        np_h = len(tile_win) - p0                   # tiles in this phase
        if np_h == 0:
            continue
        n_chunks = -(-np_h // CMAX)
        splits = np.linspace(p0, p0 + np_h, n_chunks + 1).astype(np.int64)
        for a, b in zip(splits[:-1], splits[1:]):
            if b > a:
                runs.append((int(a), int(b - a), h))
    tile_win = np.asarray(tile_win)
    assert len(tile_win) == T
    max_run = max(nt for _, nt, _ in runs)

    # per-window first/last tile within each phase (-1 if none)
    wfirst = np.full((2, NW), -1, np.int64)
    wlast = np.full((2, NW), -1, np.int64)
    for h in (0, 1):
        for w in range(NW):
            if n_th[h, w] > 0:
                wfirst[h, w] = win_tile_base[h, w]
                wlast[h, w] = win_tile_base[h, w] + n_th[h, w] - 1

    # per-core edge slot arrays
    idx_flat = np.zeros((N_CORES, T * WSZ), np.int16)
    vm8 = np.zeros((N_CORES, WSZ, T * WSZ), ml_dtypes.float8_e4m3)
    for c in range(N_CORES):
        for h in (0, 1):
            for w in range(NW):
                gidx = (c * 2 + h) * NW + w
                e0, e1 = bounds[gidx], bounds[gidx + 1]
                n = e1 - e0
                if n == 0:
                    continue
                sel = order[e0:e1]
                base = win_tile_base[h, w] * WSZ
                pos = base + np.arange(n)
                idx_flat[c, pos] = idx16[sel].astype(np.int16)
                tt = pos // WSZ
                pp = pos % WSZ
                vm8[c, pp, tt * WSZ + col[sel]] = 1.0

    # wrap indices for dma_gather: [128, T*8] int16,
    # arr[p, t*8 + cc] = idx[t*128 + cc*16 + (p % 16)]
    w16 = idx_flat.reshape(N_CORES, T, 8, 16).transpose(0, 3, 1, 2).reshape(
        N_CORES, 16, T * 8)
    idx_arr = np.tile(w16, (1, 8, 1))               # [N_CORES, 128, T*8]

    # per-core epilogue scale layouts
    dinv_pad = np.ones(N_CORES * NPC, np.float32)
    sqdeg_pad = np.ones(N_CORES * NPC, np.float32)
    dinv_pad[:N_NODES] = dinv
    sqdeg_pad[:N_NODES] = sqdeg
    dinv_own = dinv_pad.reshape(N_CORES, NW, WSZ).transpose(0, 2, 1).copy()
    sqdeg_own = sqdeg_pad.reshape(N_CORES, 1, NPC).astype(ml_dtypes.bfloat16)

    sched = dict(T=T, runs=runs, tile_win=tile_win, n_th=n_th,
                 wfirst=wfirst, wlast=wlast, max_run=max_run)
    return idx_arr, vm8, dinv_own, sqdeg_own, sched


def _build_program(sched):
    import os
    VAR = set(os.environ.get("KVAR", "").split(","))
    import concourse.bass as bass
    import concourse.bacc as bacc
    import concourse.tile as tile
    from concourse import mybir

    f32 = mybir.dt.float32
    bf16 = mybir.dt.bfloat16
    fp8 = mybir.dt.float8e4
    i16 = mybir.dt.int16

    T = sched["T"]
    runs = sched["runs"]
    tile_win = sched["tile_win"]
    n_th = sched["n_th"]
    wfirst = sched["wfirst"]
    wlast = sched["wlast"]
    max_run = sched["max_run"]

    nsq = 4
    nc = bacc.Bacc("TRN2", target_bir_lowering=False, debug=False,
                   num_devices=N_CORES, num_swdge_queues=nsq)

    xT_own = nc.dram_tensor("xT_own", [64, NPC], bf16, kind="ExternalInput")
    src_idx = nc.dram_tensor("src_idx", [128, T * 8], i16, kind="ExternalInput")
    vm_in = nc.dram_tensor("vm8", [128, T * 128], fp8, kind="ExternalInput")
    dinv_in = nc.dram_tensor("dinv_own", [128, NW], f32, kind="ExternalInput")
    sqdeg_in = nc.dram_tensor("sqdeg_own", [1, NPC], bf16, kind="ExternalInput")
    Wmat = nc.dram_tensor("Wmat", [3, 64, 64], bf16, kind="ExternalInput")
    bias_in = nc.dram_tensor("bias_r", [3, 1, 64], bf16, kind="ExternalInput")
    ident_in = nc.dram_tensor("ident", [128, 128], bf16, kind="ExternalInput")
    agents = nc.dram_tensor("agents_out", [3, NW * 32, 64], bf16,
                            kind="ExternalOutput")

    with tile.TileContext(nc) as tc:
        with (
            tc.tile_pool(name="const", bufs=1) as constp,
            tc.tile_pool(name="hT", bufs=2) as hTp,
            tc.tile_pool(name="xws", bufs=2) as xwsp,
            tc.tile_pool(name="acc", bufs=2) as accp,
            tc.tile_pool(name="msg", bufs=6) as msgp,
            tc.tile_pool(name="vm", bufs=6) as vmp,
            tc.tile_pool(name="small", bufs=4) as smallp,
            tc.tile_pool(name="ps_seg", bufs=3, space="PSUM") as ps_seg,
            tc.tile_pool(name="ps_xw", bufs=2, space="PSUM") as ps_xw,
            tc.tile_pool(name="ps_tr", bufs=2, space="PSUM") as ps_tr,
            tc.tile_pool(name="dram_ag", bufs=1, space="DRAM") as dram_ag,
            tc.tile_pool(name="dram_xw", bufs=1, space="DRAM") as dram_xw,
        ):
            meta_idx = constp.tile([128, T * 8], i16)
            nc.sync.dma_start(out=meta_idx[:], in_=src_idx[:, :])
            dinv_t = constp.tile([128, NW], f32)
            sqdeg_t = constp.tile([1, NPC], bf16)
            ident_t = constp.tile([128, 128], bf16)
            nc.sync.dma_start(out=dinv_t[:], in_=dinv_in[:, :])
            nc.sync.dma_start(out=sqdeg_t[:], in_=sqdeg_in[:, :])
            nc.sync.dma_start(out=ident_t[:], in_=ident_in[:, :])
            w_tiles = []
            b_tiles = []
            for l in range(3):
                wt = constp.tile([64, 64], bf16, name=f"w{l}")
                bt = constp.tile([1, 64], bf16, name=f"b{l}")
                nc.sync.dma_start(out=wt[:], in_=Wmat[l, :, :])
                nc.sync.dma_start(out=bt[:], in_=bias_in[l, :, :])
                w_tiles.append(wt)
                b_tiles.append(bt)

            hT_cur = hTp.tile([64, NPC], bf16, tag="hT", name="hT0")
            nc.sync.dma_start(out=hT_cur[:], in_=xT_own[:, :])

            gg = 0      # global gather counter: Tile assigns SWDGE sems as
                        # gather#%8, so queue gather#%4 keeps sem<->queue 1:1
            for l in range(3):
                # ---- own-shard linear: xw = (h_own @ W_l) * dinv_own ----
                xw_stage = xwsp.tile([128, NW * 64], bf16, tag="xws",
                                     name=f"xws{l}")
                for w in range(NW):
                    ps = ps_xw.tile([128, 64], f32, tag="psxw",
                                    name=f"psxw{l}_{w}")
                    nc.tensor.matmul(
                        out=ps[:],
                        lhsT=hT_cur[:, w * 128:(w + 1) * 128],
                        rhs=w_tiles[l][:],
                        start=True, stop=True,
                    )
                    nc.scalar.mul(out=xw_stage[:, w * 64:(w + 1) * 64],
                                  in_=ps[:], mul=dinv_t[:, w:w + 1])

                ag_t = dram_ag.tile([NPC, 128], bf16, tag="ag", name=f"ag{l}")
                nc.sync.dma_start(
                    out=ag_t[:].rearrange("(w p) f -> p w f", p=128)[:, :, 0:64],
                    in_=xw_stage[:].rearrange("p (w f) -> p w f", f=64),
                )

                xwA = dram_xw.tile([ROWS_A, 128], bf16, tag="xwA",
                                   addr_space="Shared", name=f"xwA{l}")
                xwB = dram_xw.tile([ROWS_B, 128], bf16, tag="xwB",
                                   addr_space="Shared", name=f"xwB{l}")
                if "nocoll" in VAR:
                    nc.sync.dma_start(out=xwA[0:HALF_A, :],
                                      in_=ag_t[0:HALF_A, :])
                    nc.sync.dma_start(out=xwB[0:HALF_B, :],
                                      in_=ag_t[HALF_A:NPC, :])
                else:
                    nc.gpsimd.collective_compute(
                        "AllGather",
                        mybir.AluOpType.bypass,
                        replica_groups=[list(range(N_CORES))],
                        ins=[ag_t[0:HALF_A, :].opt()],
                        outs=[xwA.opt()],
                    )
                    nc.gpsimd.collective_compute(
                        "AllGather",
                        mybir.AluOpType.bypass,
                        replica_groups=[list(range(N_CORES))],
                        ins=[ag_t[HALF_A:NPC, :].opt()],
                        outs=[xwB.opt()],
                    )

                if l < 2:
                    hT_next = hTp.tile([64, NPC], bf16, tag="hT",
                                       name=f"hT{l + 1}")
                else:
                    hT_next = None

                # parked phase-A partial sums, one [128, 64] slice per window
                acc_t = accp.tile([128, NW * 64], bf16, tag="acc",
                                  name=f"acc{l}")

                def epilogue(w, cur_ps):
                    hwin = smallp.tile([128, 64], bf16, tag="hwin",
                                       name=f"hw{l}_{w}")
                    nc.scalar.activation(
                        out=hwin[:], in_=cur_ps[:],
                        func=mybir.ActivationFunctionType.Sigmoid,
                        scale=dinv_t[:, w:w + 1],
                    )
                    nc.sync.dma_start(
                        out=agents[l, w * 32:(w + 1) * 32, :],
                        in_=hwin[0:128:4, :],
                    )
                    if hT_next is not None:
                        pt = ps_tr.tile([64, 128], bf16, tag="tr",
                                        name=f"tr{l}_{w}")
                        nc.tensor.transpose(out=pt[:], in_=hwin[:],
                                            identity=ident_t[:])
                        nc.scalar.copy(
                            out=hT_next[:, w * 128:(w + 1) * 128],
                            in_=pt[:],
                        )

                # ---- gather + two-phase windowed segment-sum ----
                win_ps = {}
                for r, (t0, nt, h) in enumerate(runs):
                    msg = msgp.tile([128, max_run, 128], bf16, tag="msg",
                                    name=f"msg{l}_{r}")
                    vm_t = vmp.tile([128, max_run * 128], fp8, tag="vm",
                                    name=f"vm{l}_{r}")
                    nc.sync.dma_start(
                        out=vm_t[:, :nt * 128],
                        in_=vm_in[:, t0 * 128:(t0 + nt) * 128])
                    if "nogather" not in VAR:
                        # single_packet amortizes SDMA per-packet overhead but
                        # caps a call at 64 descs/engine = 8 tiles; sub-gathers
                        # share the run's queue so buffer/queue sems align
                        sp = "nosp" not in VAR
                        step = 8 if sp else nt
                        for s0 in range(0, nt, step):
                            sn = min(step, nt - s0)
                            nc.gpsimd.dma_gather(
                                out_ap=msg[:, s0:s0 + sn, :],
                                in_ap=(xwA[:] if h == 0 else xwB[:]),
                                idxs_ap=meta_idx[:, (t0 + s0) * 8:
                                                 (t0 + s0 + sn) * 8],
                                num_idxs=sn * 128,
                                num_idxs_reg=sn * 128,
                                elem_size=128,
                                single_packet=sp,
                                queue_num=(gg % nsq),
                            )
                            gg += 1
                    for j in range(nt):
                        t = t0 + j
                        w = int(tile_win[t])
                        if t == wfirst[h, w]:
                            cur = ps_seg.tile([128, 64], f32, tag="seg",
                                              name=f"seg{l}_{h}_{w}")
                            win_ps[w] = cur
                            if h == 0 or wfirst[0, w] < 0:
                                # open with bias: psum = sqrt(deg) x bias
                                nc.tensor.matmul(
                                    out=cur[:],
                                    lhsT=sqdeg_t[:, w * 128:(w + 1) * 128],
                                    rhs=b_tiles[l][:],
                                    start=True, stop=False,
                                )
                                # self-loop term: += dinv[i] * xw[i]
                                nc.tensor.matmul(
                                    out=cur[:],
                                    lhsT=ident_t[:],
                                    rhs=xw_stage[:, w * 64:(w + 1) * 64],
                                    start=False, stop=False,
                                )
                            else:
                                # re-inject parked phase-A sum
                                nc.tensor.matmul(
                                    out=cur[:],
                                    lhsT=ident_t[:],
                                    rhs=acc_t[:, w * 64:(w + 1) * 64],
                                    start=True, stop=False,
                                )
                        cur_ps = win_ps[w]
                        last = (t == wlast[h, w])
                        if "nomm" not in VAR:
                            nc.tensor.matmul(
                                out=cur_ps[:],
                                lhsT=vm_t[:, j * 128:(j + 1) * 128],
                                rhs=msg[:, j, 0:64],
                                start=False, stop=last,
                            )
                        elif last:
                            nc.scalar.copy(out=cur_ps[:], in_=cur_ps[:])
                        if last:
                            if h == 0 and wlast[1, w] >= 0:
                                # park phase-A sum in SBUF
                                nc.scalar.copy(
                                    out=acc_t[:, w * 64:(w + 1) * 64],
                                    in_=cur_ps[:])
                            else:
                                epilogue(w, cur_ps)

                # windows with no tiles at all (pad safety)
                for w in range(NW):
                    if wlast[0, w] < 0 and wlast[1, w] < 0:
                        cur = ps_seg.tile([128, 64], f32, tag="seg",
                                          name=f"segz{l}_{w}")
                        nc.tensor.matmul(
                            out=cur[:],
                            lhsT=sqdeg_t[:, w * 128:(w + 1) * 128],
                            rhs=b_tiles[l][:],
                            start=True, stop=False,
                        )
                        nc.tensor.matmul(
                            out=cur[:],
                            lhsT=ident_t[:],
                            rhs=xw_stage[:, w * 64:(w + 1) * 64],
                            start=False, stop=True,
                        )
                        epilogue(w, cur)

                hT_cur = hT_next

    nc.compile()
    return nc


def kernel(**inputs):
    from concourse import bass_utils

    x = np.asarray(inputs["x"], dtype=np.float32)
    edge_index = np.asarray(inputs["edge_index"])
    agent_idx = np.asarray(inputs["agent_idx"], dtype=np.int64)
    Ws = [np.asarray(inputs[f"W{i}"], dtype=np.float32) for i in range(3)]
    bs = [np.asarray(inputs[f"b{i}"], dtype=np.float32) for i in range(3)]

    idx_arr, vm8, dinv_own, sqdeg_own, sched = _preprocess(edge_index)

    nc = _build_program(sched)

    xpad = np.zeros((N_CORES * NPC, D), np.float32)
    xpad[:N_NODES] = x
    Wstack = np.ascontiguousarray(
        np.stack(Ws)).astype(ml_dtypes.bfloat16)
    bias_stack = np.ascontiguousarray(
        np.stack([b[None, :] for b in bs])).astype(ml_dtypes.bfloat16)
    ident = np.eye(128, dtype=ml_dtypes.bfloat16)

    in_maps = []
    for c in range(N_CORES):
        in_maps.append({
            "xT_own": np.ascontiguousarray(
                xpad[c * NPC:(c + 1) * NPC].T).astype(ml_dtypes.bfloat16),
            "src_idx": np.ascontiguousarray(idx_arr[c]),
            "vm8": np.ascontiguousarray(vm8[c]),
            "dinv_own": np.ascontiguousarray(dinv_own[c]),
            "sqdeg_own": np.ascontiguousarray(sqdeg_own[c]),
            "Wmat": Wstack,
            "bias_r": bias_stack,
            "ident": ident,
        })

    res = bass_utils.run_bass_kernel_spmd(
        nc, in_maps, core_ids=list(range(N_CORES)))

    taps = np.stack([np.asarray(res.results[c]["agents_out"])
                     .astype(np.float32) for c in range(N_CORES)])
    # taps[c, l, r, :] = h_l for node (c*NPC + 4*r)
    n_agents = agent_idx.shape[0]
    out = np.empty((n_agents, 3 * D), np.float32)
    c_of = agent_idx // NPC
    r_of = (agent_idx % NPC) // 4
    for l in range(3):
        out[:, l * D:(l + 1) * D] = taps[c_of, l, r_of, :]
    return out


# revision 42
# speedup vs baseline: 1.1015x; 1.0514x over previous
"""3-layer GCN (PyG GCNConv-style) on 8 Trainium2 NeuronCores — v2.

Strategy (graph/data parallel; nodes sharded by destination core):
  - Nodes partitioned contiguously: 6272 per core (49 windows x 128). Edges
    (incl. host-added self-loops) are owned by the core owning their dst.
  - Normalization is folded away: the gathered xw table holds
    dinv[src] * (h @ W) rows, and the window epilogue applies the dinv[dst]
    factor as the per-partition `scale` of the sigmoid activation. The bias
    is injected into PSUM via a K=1 rank-1 matmul with a sqrt(deg[dst])
    column so it survives the later dinv[dst] scaling.
  - The per-edge one-hot scatter matrices are graph-static: built ONCE on
    the host in fp8 (exact for 0/1) and streamed from DRAM each layer,
    freeing the Vector engine entirely (v1 spent 85% of the span there).
  - Gathers use int16 indices, so the 50176-row xw table is split into
    half-tables A (local row < 3200, 25600 rows) and B (24576 rows). Each
    layer runs two phases: phase A processes every window's A-half tiles
    (window-major, one live PSUM bank, parked to SBUF in bf16 at window
    close), phase B re-injects the parked sum via an identity matmul and
    finishes the window (sigmoid epilogue, agent tap, PE transpose into the
    next layer's hT). Phases are split into CMAX-tile gather chunks, each
    issued as 8-tile sub-gathers (single_packet coalescing caps a call at
    64 descs/engine) spread round-robin over the 4 SWDGE queues so all
    four Q7 core pairs generate descriptors in parallel (~7.4ns/index each).
  - Per layer: 49 own-shard bf16 matmuls -> ScalarE evac (x dinv, cast bf16)
    -> one DMA into the padded [6272, 128]-bf16 shard -> two AllGathers
    (A-half first so phase-A gathers start sooner).

Host-side work: graph preprocessing (degrees, edge layout, one-hot tiles)
and final output assembly.
"""

import sys

sys.path.insert(0, "/opt/trn_rl_repo")

import numpy as np
import ml_dtypes

N_NODES = 50000
D = 64
N_CORES = 8
WSZ = 128               # dst-window size (PSUM partition dim)
NW = 49                 # windows per core
NPC = NW * WSZ          # 6272 padded nodes per core (50176 total >= 50000)
HALF_A = 3200           # local rows < HALF_A -> table A (25 windows' rows)
HALF_B = NPC - HALF_A   # 3072 rows -> table B
ROWS_A = N_CORES * HALF_A   # 25600 (< 32767, int16-addressable)
ROWS_B = N_CORES * HALF_B   # 24576
CMAX = 48               # max tiles per gather chunk; small chunks + deep
                        # buffering let 4 queues (= Q7 core pairs) gen in parallel


def _preprocess(edge_index):
    """Edge layout + one-hot scatter tiles.

    Tile stream: [phase A: w0..w48, each window's A-half tiles]
                 [phase B: w0..w48, each window's B-half tiles].
    SPMD: tile counts per (window, half) are maxed over cores; padded slots
    get all-zero one-hot rows so they contribute nothing.
    """
    src = np.asarray(edge_index[0], dtype=np.int64)
    dst = np.asarray(edge_index[1], dtype=np.int64)

    deg = np.bincount(dst, minlength=N_NODES).astype(np.float32) + 1.0
    dinv = (1.0 / np.sqrt(deg)).astype(np.float32)
    sqdeg = np.sqrt(deg).astype(np.float32)

    # self-loops are NOT gathered: their dinv[i]*xw[i] rows live in the
    # local xw_stage and are added via one identity matmul per window
    s_all = src
    d_all = dst

    core = d_all // NPC
    local = d_all - core * NPC
    win = local // WSZ
    col = local % WSZ

    s_core = s_all // NPC
    s_loc = s_all - s_core * NPC
    half = (s_loc >= HALF_A).astype(np.int64)
    idx16 = np.where(half == 0, s_core * HALF_A + s_loc,
                     s_core * HALF_B + (s_loc - HALF_A))

    # group edges by (core, half, win)
    key = (core * 2 + half) * NW + win
    nkey = N_CORES * 2 * NW
    order = np.argsort(key, kind="stable")
    key_sorted = key[order]
    bounds = np.searchsorted(key_sorted, np.arange(nkey + 1))
    cnt = (bounds[1:] - bounds[:-1]).reshape(N_CORES, 2, NW)

    # uniform tiles per (half, win), maxed over cores
    n_th = -(-cnt.max(axis=0) // WSZ)               # [2, NW]
    T = int(n_th.sum())

    # tile stream + gather chunks (runs)
    tile_win = []
    runs = []                                       # (t0, nt, half)
    win_tile_base = np.zeros((2, NW), np.int64)
    for h in (0, 1):
        p0 = len(tile_win)
        for w in range(NW):
            win_tile_base[h, w] = len(tile_win)
            tile_win += [w] * int(n_th[h, w])
        np_h = len(tile_win) - p0                   # tiles in this phase
        if np_h == 0:
            continue
        n_chunks = -(-np_h // CMAX)
        splits = np.linspace(p0, p0 + np_h, n_chunks + 1).astype(np.int64)
        for a, b in zip(splits[:-1], splits[1:]):
            if b > a:
                runs.append((int(a), int(b - a), h))
    tile_win = np.asarray(tile_win)
    assert len(tile_win) == T
    max_run = max(nt for _, nt, _ in runs)

    # per-window first/last tile within each phase (-1 if none)
    wfirst = np.full((2, NW), -1, np.int64)
    wlast = np.full((2, NW), -1, np.int64)
    for h in (0, 1):
        for w in range(NW):
            if n_th[h, w] > 0:
                wfirst[h, w] = win_tile_base[h, w]
                wlast[h, w] = win_tile_base[h, w] + n_th[h, w] - 1

    # per-core edge slot arrays
    idx_flat = np.zeros((N_CORES, T * WSZ), np.int16)
    vm8 = np.zeros((N_CORES, WSZ, T * WSZ), ml_dtypes.float8_e4m3)
    for c in range(N_CORES):
        for h in (0, 1):
            for w in range(NW):
                gidx = (c * 2 + h) * NW + w
                e0, e1 = bounds[gidx], bounds[gidx + 1]
                n = e1 - e0
                if n == 0:
                    continue
                sel = order[e0:e1]
                base = win_tile_base[h, w] * WSZ
                pos = base + np.arange(n)
                idx_flat[c, pos] = idx16[sel].astype(np.int16)
                tt = pos // WSZ
                pp = pos % WSZ
                vm8[c, pp, tt * WSZ + col[sel]] = 1.0

    # wrap indices for dma_gather: [128, T*8] int16,
    # arr[p, t*8 + cc] = idx[t*128 + cc*16 + (p % 16)]
    w16 = idx_flat.reshape(N_CORES, T, 8, 16).transpose(0, 3, 1, 2).reshape(
        N_CORES, 16, T * 8)
    idx_arr = np.tile(w16, (1, 8, 1))               # [N_CORES, 128, T*8]

    # per-core epilogue scale layouts
    dinv_pad = np.ones(N_CORES * NPC, np.float32)
    sqdeg_pad = np.ones(N_CORES * NPC, np.float32)
    dinv_pad[:N_NODES] = dinv
    sqdeg_pad[:N_NODES] = sqdeg
    dinv_own = dinv_pad.reshape(N_CORES, NW, WSZ).transpose(0, 2, 1).copy()
    sqdeg_own = sqdeg_pad.reshape(N_CORES, 1, NPC).astype(ml_dtypes.bfloat16)

    sched = dict(T=T, runs=runs, tile_win=tile_win, n_th=n_th,
                 wfirst=wfirst, wlast=wlast, max_run=max_run)
    return idx_arr, vm8, dinv_own, sqdeg_own, sched


def _build_program(sched):
    import os
    VAR = set(os.environ.get("KVAR", "").split(","))
    import concourse.bass as bass
    import concourse.bacc as bacc
    import concourse.tile as tile
    from concourse import mybir

    f32 = mybir.dt.float32
    bf16 = mybir.dt.bfloat16
    fp8 = mybir.dt.float8e4
    i16 = mybir.dt.int16

    T = sched["T"]
    runs = sched["runs"]
    tile_win = sched["tile_win"]
    n_th = sched["n_th"]
    wfirst = sched["wfirst"]
    wlast = sched["wlast"]
    max_run = sched["max_run"]

    nsq = 4
    nc = bacc.Bacc("TRN2", target_bir_lowering=False, debug=False,
                   num_devices=N_CORES, num_swdge_queues=nsq)

    xT_own = nc.dram_tensor("xT_own", [64, NPC], bf16, kind="ExternalInput")
    src_idx = nc.dram_tensor("src_idx", [128, T * 8], i16, kind="ExternalInput")
    vm_in = nc.dram_tensor("vm8", [128, T * 128], fp8, kind="ExternalInput")
    dinv_in = nc.dram_tensor("dinv_own", [128, NW], f32, kind="ExternalInput")
    sqdeg_in = nc.dram_tensor("sqdeg_own", [1, NPC], bf16, kind="ExternalInput")
    Wmat = nc.dram_tensor("Wmat", [3, 64, 64], bf16, kind="ExternalInput")
    bias_in = nc.dram_tensor("bias_r", [3, 1, 64], bf16, kind="ExternalInput")
    ident_in = nc.dram_tensor("ident", [128, 128], bf16, kind="ExternalInput")
    agents = nc.dram_tensor("agents_out", [3, NW * 32, 64], bf16,
                            kind="ExternalOutput")

    with tile.TileContext(nc) as tc:
        with (
            tc.tile_pool(name="const", bufs=1) as constp,
            tc.tile_pool(name="hT", bufs=2) as hTp,
            tc.tile_pool(name="xws", bufs=2) as xwsp,
            tc.tile_pool(name="acc", bufs=2) as accp,
            tc.tile_pool(name="msg", bufs=6) as msgp,
            tc.tile_pool(name="vm", bufs=6) as vmp,
            tc.tile_pool(name="small", bufs=4) as smallp,
            tc.tile_pool(name="ps_seg", bufs=3, space="PSUM") as ps_seg,
            tc.tile_pool(name="ps_xw", bufs=2, space="PSUM") as ps_xw,
            tc.tile_pool(name="ps_tr", bufs=2, space="PSUM") as ps_tr,
            tc.tile_pool(name="dram_ag", bufs=1, space="DRAM") as dram_ag,
            tc.tile_pool(name="dram_xw", bufs=1, space="DRAM") as dram_xw,
        ):
            meta_idx = constp.tile([128, T * 8], i16)
            nc.sync.dma_start(out=meta_idx[:], in_=src_idx[:, :])
            dinv_t = constp.tile([128, NW], f32)
            sqdeg_t = constp.tile([1, NPC], bf16)
            ident_t = constp.tile([128, 128], bf16)
            nc.sync.dma_start(out=dinv_t[:], in_=dinv_in[:, :])
            nc.sync.dma_start(out=sqdeg_t[:], in_=sqdeg_in[:, :])
            nc.sync.dma_start(out=ident_t[:], in_=ident_in[:, :])
            w_tiles = []
            b_tiles = []
            for l in range(3):
                wt = constp.tile([64, 64], bf16, name=f"w{l}")
                bt = constp.tile([1, 64], bf16, name=f"b{l}")
                nc.sync.dma_start(out=wt[:], in_=Wmat[l, :, :])
                nc.sync.dma_start(out=bt[:], in_=bias_in[l, :, :])
                w_tiles.append(wt)
                b_tiles.append(bt)

            hT_cur = hTp.tile([64, NPC], bf16, tag="hT", name="hT0")
            nc.sync.dma_start(out=hT_cur[:], in_=xT_own[:, :])

            gg = 0      # global gather counter: Tile assigns SWDGE sems as
                        # gather#%8, so queue gather#%4 keeps sem<->queue 1:1
            NWLO = HALF_A // WSZ        # 25 windows feed table A
            hTs = {0: hT_cur}
            xws, ags, xwAs, xwBs = {}, {}, {}, {}

            def emit_xw_half(l, lo):
                """xw = (h @ W_l) * dinv for windows [0,25) or [25,49), plus
                the matching shard store + AllGather. The lo half of layer
                l+1 is emitted inside layer l so AG_a overlaps its PE tail."""
                if l not in xws:
                    xws[l] = xwsp.tile([128, NW * 64], bf16, tag="xws",
                                       name=f"xws{l}")
                    ags[l] = dram_ag.tile([NPC, 128], bf16, tag="ag",
                                          name=f"ag{l}")
                    xwAs[l] = dram_xw.tile([ROWS_A, 128], bf16, tag="xwA",
                                           addr_space="Shared",
                                           name=f"xwA{l}")
                    xwBs[l] = dram_xw.tile([ROWS_B, 128], bf16, tag="xwB",
                                           addr_space="Shared",
                                           name=f"xwB{l}")
                ws = range(0, NWLO) if lo else range(NWLO, NW)
                for w in ws:
                    ps = ps_xw.tile([128, 64], f32, tag="psxw",
                                    name=f"psxw{l}_{w}")
                    nc.tensor.matmul(
                        out=ps[:],
                        lhsT=hTs[l][:, w * 128:(w + 1) * 128],
                        rhs=w_tiles[l][:],
                        start=True, stop=True,
                    )
                    nc.scalar.mul(out=xws[l][:, w * 64:(w + 1) * 64],
                                  in_=ps[:], mul=dinv_t[:, w:w + 1])
                r0, r1 = (0, HALF_A) if lo else (HALF_A, NPC)
                nc.sync.dma_start(
                    out=ags[l][r0:r1, :].rearrange(
                        "(w p) f -> p w f", p=128)[:, :, 0:64],
                    in_=xws[l][:, r0 // 2:r1 // 2].rearrange(
                        "p (w f) -> p w f", f=64),
                )
                dst = xwAs[l] if lo else xwBs[l]
                if "nocoll" in VAR:
                    nc.sync.dma_start(out=dst[0:r1 - r0, :],
                                      in_=ags[l][r0:r1, :])
                else:
                    nc.gpsimd.collective_compute(
                        "AllGather",
                        mybir.AluOpType.bypass,
                        replica_groups=[list(range(N_CORES))],
                        ins=[ags[l][r0:r1, :].opt()],
                        outs=[dst.opt()],
                    )

            for l in range(3):
                if l == 0 or "nohoist" in VAR:
                    emit_xw_half(l, lo=True)
                emit_xw_half(l, lo=False)
                xw_stage = xws[l]
                xwA, xwB = xwAs[l], xwBs[l]

                if l < 2:
                    hT_next = hTp.tile([64, NPC], bf16, tag="hT",
                                       name=f"hT{l + 1}")
                    hTs[l + 1] = hT_next
                else:
                    hT_next = None

                # parked phase-A partial sums, one [128, 64] slice per window
                acc_t = accp.tile([128, NW * 64], bf16, tag="acc",
                                  name=f"acc{l}")

                def epilogue(w, cur_ps):
                    hwin = smallp.tile([128, 64], bf16, tag="hwin",
                                       name=f"hw{l}_{w}")
                    nc.scalar.activation(
                        out=hwin[:], in_=cur_ps[:],
                        func=mybir.ActivationFunctionType.Sigmoid,
                        scale=dinv_t[:, w:w + 1],
                    )
                    nc.sync.dma_start(
                        out=agents[l, w * 32:(w + 1) * 32, :],
                        in_=hwin[0:128:4, :],
                    )
                    if hT_next is not None:
                        pt = ps_tr.tile([64, 128], bf16, tag="tr",
                                        name=f"tr{l}_{w}")
                        nc.tensor.transpose(out=pt[:], in_=hwin[:],
                                            identity=ident_t[:])
                        nc.scalar.copy(
                            out=hT_next[:, w * 128:(w + 1) * 128],
                            in_=pt[:],
                        )

                # ---- gather + two-phase windowed segment-sum ----
                win_ps = {}
                for r, (t0, nt, h) in enumerate(runs):
                    msg = msgp.tile([128, max_run, 128], bf16, tag="msg",
                                    name=f"msg{l}_{r}")
                    vm_t = vmp.tile([128, max_run * 128], fp8, tag="vm",
                                    name=f"vm{l}_{r}")
                    nc.sync.dma_start(
                        out=vm_t[:, :nt * 128],
                        in_=vm_in[:, t0 * 128:(t0 + nt) * 128])
                    if "nogather" not in VAR:
                        # single_packet amortizes SDMA per-packet overhead but
                        # caps a call at 64 descs/engine = 8 tiles; sub-gathers
                        # share the run's queue so buffer/queue sems align
                        sp = "nosp" not in VAR
                        step = 8 if sp else nt
                        for s0 in range(0, nt, step):
                            sn = min(step, nt - s0)
                            nc.gpsimd.dma_gather(
                                out_ap=msg[:, s0:s0 + sn, :],
                                in_ap=(xwA[:] if h == 0 else xwB[:]),
                                idxs_ap=meta_idx[:, (t0 + s0) * 8:
                                                 (t0 + s0 + sn) * 8],
                                num_idxs=sn * 128,
                                num_idxs_reg=sn * 128,
                                elem_size=128,
                                single_packet=sp,
                                queue_num=(gg % nsq),
                            )
                            gg += 1
                    for j in range(nt):
                        t = t0 + j
                        w = int(tile_win[t])
                        if t == wfirst[h, w]:
                            cur = ps_seg.tile([128, 64], f32, tag="seg",
                                              name=f"seg{l}_{h}_{w}")
                            win_ps[w] = cur
                            if h == 0 or wfirst[0, w] < 0:
                                # open with bias: psum = sqrt(deg) x bias
                                nc.tensor.matmul(
                                    out=cur[:],
                                    lhsT=sqdeg_t[:, w * 128:(w + 1) * 128],
                                    rhs=b_tiles[l][:],
                                    start=True, stop=False,
                                )
                                # self-loop term: += dinv[i] * xw[i]
                                nc.tensor.matmul(
                                    out=cur[:],
                                    lhsT=ident_t[:],
                                    rhs=xw_stage[:, w * 64:(w + 1) * 64],
                                    start=False, stop=False,
                                )
                            else:
                                # re-inject parked phase-A sum
                                nc.tensor.matmul(
                                    out=cur[:],
                                    lhsT=ident_t[:],
                                    rhs=acc_t[:, w * 64:(w + 1) * 64],
                                    start=True, stop=False,
                                )
                        cur_ps = win_ps[w]
                        last = (t == wlast[h, w])
                        if "nomm" not in VAR:
                            nc.tensor.matmul(
                                out=cur_ps[:],
                                lhsT=vm_t[:, j * 128:(j + 1) * 128],
                                rhs=msg[:, j, 0:64],
                                start=False, stop=last,
                            )
                        elif last:
                            nc.scalar.copy(out=cur_ps[:], in_=cur_ps[:])
                        if last:
                            if h == 0 and wlast[1, w] >= 0:
                                # park phase-A sum in SBUF
                                nc.scalar.copy(
                                    out=acc_t[:, w * 64:(w + 1) * 64],
                                    in_=cur_ps[:])
                            else:
                                epilogue(w, cur_ps)

                # windows with no tiles at all (pad safety)
                for w in range(NW):
                    if wlast[0, w] < 0 and wlast[1, w] < 0:
                        cur = ps_seg.tile([128, 64], f32, tag="seg",
                                          name=f"segz{l}_{w}")
                        nc.tensor.matmul(
                            out=cur[:],
                            lhsT=sqdeg_t[:, w * 128:(w + 1) * 128],
                            rhs=b_tiles[l][:],
                            start=True, stop=False,
                        )
                        nc.tensor.matmul(
                            out=cur[:],
                            lhsT=ident_t[:],
                            rhs=xw_stage[:, w * 64:(w + 1) * 64],
                            start=False, stop=True,
                        )
                        epilogue(w, cur)

                if l < 2 and "nohoist" not in VAR:
                    # hoist next layer's lo-half xw + AG_a into this layer's
                    # PE/epilogue tail (windows 0-24's h are already final)
                    emit_xw_half(l + 1, lo=True)

                hT_cur = hT_next

    nc.compile()
    return nc


def kernel(**inputs):
    from concourse import bass_utils

    x = np.asarray(inputs["x"], dtype=np.float32)
    edge_index = np.asarray(inputs["edge_index"])
    agent_idx = np.asarray(inputs["agent_idx"], dtype=np.int64)
    Ws = [np.asarray(inputs[f"W{i}"], dtype=np.float32) for i in range(3)]
    bs = [np.asarray(inputs[f"b{i}"], dtype=np.float32) for i in range(3)]

    idx_arr, vm8, dinv_own, sqdeg_own, sched = _preprocess(edge_index)

    nc = _build_program(sched)

    xpad = np.zeros((N_CORES * NPC, D), np.float32)
    xpad[:N_NODES] = x
    Wstack = np.ascontiguousarray(
        np.stack(Ws)).astype(ml_dtypes.bfloat16)
    bias_stack = np.ascontiguousarray(
        np.stack([b[None, :] for b in bs])).astype(ml_dtypes.bfloat16)
    ident = np.eye(128, dtype=ml_dtypes.bfloat16)

    in_maps = []
    for c in range(N_CORES):
        in_maps.append({
            "xT_own": np.ascontiguousarray(
                xpad[c * NPC:(c + 1) * NPC].T).astype(ml_dtypes.bfloat16),
            "src_idx": np.ascontiguousarray(idx_arr[c]),
            "vm8": np.ascontiguousarray(vm8[c]),
            "dinv_own": np.ascontiguousarray(dinv_own[c]),
            "sqdeg_own": np.ascontiguousarray(sqdeg_own[c]),
            "Wmat": Wstack,
            "bias_r": bias_stack,
            "ident": ident,
        })

    res = bass_utils.run_bass_kernel_spmd(
        nc, in_maps, core_ids=list(range(N_CORES)))

    taps = np.stack([np.asarray(res.results[c]["agents_out"])
                     .astype(np.float32) for c in range(N_CORES)])
    # taps[c, l, r, :] = h_l for node (c*NPC + 4*r)
    n_agents = agent_idx.shape[0]
    out = np.empty((n_agents, 3 * D), np.float32)
    c_of = agent_idx // NPC
    r_of = (agent_idx % NPC) // 4
    for l in range(3):
        out[:, l * D:(l + 1) * D] = taps[c_of, l, r_of, :]
    return out


# revision 45
# speedup vs baseline: 1.1218x; 1.0184x over previous
"""3-layer GCN (PyG GCNConv-style) on 8 Trainium2 NeuronCores — v2.

Strategy (graph/data parallel; nodes sharded by destination core):
  - Nodes partitioned contiguously: 6272 per core (49 windows x 128). Edges
    (incl. host-added self-loops) are owned by the core owning their dst.
  - Normalization is folded away: the gathered xw table holds
    dinv[src] * (h @ W) rows, and the window epilogue applies the dinv[dst]
    factor as the per-partition `scale` of the sigmoid activation. The bias
    is injected into PSUM via a K=1 rank-1 matmul with a sqrt(deg[dst])
    column so it survives the later dinv[dst] scaling.
  - The per-edge one-hot scatter matrices are graph-static: built ONCE on
    the host in fp8 (exact for 0/1) and streamed from DRAM each layer,
    freeing the Vector engine entirely (v1 spent 85% of the span there).
  - Gathers use int16 indices, so the 50176-row xw table is split into
    half-tables A (local row < 3200, 25600 rows) and B (24576 rows). Each
    layer runs two phases: phase A processes every window's A-half tiles
    (window-major, one live PSUM bank, parked to SBUF in bf16 at window
    close), phase B re-injects the parked sum via an identity matmul and
    finishes the window (sigmoid epilogue, agent tap, PE transpose into the
    next layer's hT). Phases are split into CMAX-tile gather chunks, each
    issued as 8-tile sub-gathers (single_packet coalescing caps a call at
    64 descs/engine) spread round-robin over the 4 SWDGE queues so all
    four Q7 core pairs generate descriptors in parallel (~7.4ns/index each).
  - Per layer: 49 own-shard bf16 matmuls -> ScalarE evac (x dinv, cast bf16)
    -> one DMA into the padded [6272, 128]-bf16 shard -> two AllGathers
    (A-half first so phase-A gathers start sooner).

Host-side work: graph preprocessing (degrees, edge layout, one-hot tiles)
and final output assembly.
"""

import sys

sys.path.insert(0, "/opt/trn_rl_repo")

import numpy as np
import ml_dtypes

N_NODES = 50000
D = 64
N_CORES = 8
WSZ = 128               # dst-window size (PSUM partition dim)
NW = 49                 # windows per core
NPC = NW * WSZ          # 6272 padded nodes per core (50176 total >= 50000)
HALF_A = 3200           # local rows < HALF_A -> table A (25 windows' rows)
HALF_B = NPC - HALF_A   # 3072 rows -> table B
ROWS_A = N_CORES * HALF_A   # 25600 (< 32767, int16-addressable)
ROWS_B = N_CORES * HALF_B   # 24576
CMAX = 48               # max tiles per gather chunk; small chunks + deep
                        # buffering let 4 queues (= Q7 core pairs) gen in parallel


def _preprocess(edge_index):
    """Edge layout + one-hot scatter tiles.

    Tile stream: [phase A: w0..w48, each window's A-half tiles]
                 [phase B: w0..w48, each window's B-half tiles].
    SPMD: tile counts per (window, half) are maxed over cores; padded slots
    get all-zero one-hot rows so they contribute nothing.
    """
    src = np.asarray(edge_index[0], dtype=np.int64)
    dst = np.asarray(edge_index[1], dtype=np.int64)

    deg = np.bincount(dst, minlength=N_NODES).astype(np.float32) + 1.0
    dinv = (1.0 / np.sqrt(deg)).astype(np.float32)
    sqdeg = np.sqrt(deg).astype(np.float32)

    # self-loops are NOT gathered: their dinv[i]*xw[i] rows live in the
    # local xw_stage and are added via one identity matmul per window
    s_all = src
    d_all = dst

    core = d_all // NPC
    local = d_all - core * NPC
    win = local // WSZ
    col = local % WSZ

    s_core = s_all // NPC
    s_loc = s_all - s_core * NPC
    half = (s_loc >= HALF_A).astype(np.int64)
    idx16 = np.where(half == 0, s_core * HALF_A + s_loc,
                     s_core * HALF_B + (s_loc - HALF_A))

    # group edges by (core, half, win)
    key = (core * 2 + half) * NW + win
    nkey = N_CORES * 2 * NW
    order = np.argsort(key, kind="stable")
    key_sorted = key[order]
    bounds = np.searchsorted(key_sorted, np.arange(nkey + 1))
    cnt = (bounds[1:] - bounds[:-1]).reshape(N_CORES, 2, NW)

    # uniform tiles per (half, win), maxed over cores
    n_th = -(-cnt.max(axis=0) // WSZ)               # [2, NW]
    T = int(n_th.sum())

    # tile stream + gather chunks (runs)
    tile_win = []
    runs = []                                       # (t0, nt, half)
    win_tile_base = np.zeros((2, NW), np.int64)
    for h in (0, 1):
        p0 = len(tile_win)
        for w in range(NW):
            win_tile_base[h, w] = len(tile_win)
            tile_win += [w] * int(n_th[h, w])
        np_h = len(tile_win) - p0                   # tiles in this phase
        if np_h == 0:
            continue
        n_chunks = -(-np_h // CMAX)
        splits = np.linspace(p0, p0 + np_h, n_chunks + 1).astype(np.int64)
        for a, b in zip(splits[:-1], splits[1:]):
            if b > a:
                runs.append((int(a), int(b - a), h))
    tile_win = np.asarray(tile_win)
    assert len(tile_win) == T
    max_run = max(nt for _, nt, _ in runs)

    # per-window first/last tile within each phase (-1 if none)
    wfirst = np.full((2, NW), -1, np.int64)
    wlast = np.full((2, NW), -1, np.int64)
    for h in (0, 1):
        for w in range(NW):
            if n_th[h, w] > 0:
                wfirst[h, w] = win_tile_base[h, w]
                wlast[h, w] = win_tile_base[h, w] + n_th[h, w] - 1

    # per-core edge slot arrays
    idx_flat = np.zeros((N_CORES, T * WSZ), np.int16)
    vm8 = np.zeros((N_CORES, WSZ, T * WSZ), ml_dtypes.float8_e4m3)
    for c in range(N_CORES):
        for h in (0, 1):
            for w in range(NW):
                gidx = (c * 2 + h) * NW + w
                e0, e1 = bounds[gidx], bounds[gidx + 1]
                n = e1 - e0
                if n == 0:
                    continue
                sel = order[e0:e1]
                base = win_tile_base[h, w] * WSZ
                pos = base + np.arange(n)
                idx_flat[c, pos] = idx16[sel].astype(np.int16)
                tt = pos // WSZ
                pp = pos % WSZ
                vm8[c, pp, tt * WSZ + col[sel]] = 1.0

    # wrap indices for dma_gather: [128, T*8] int16,
    # arr[p, t*8 + cc] = idx[t*128 + cc*16 + (p % 16)]
    w16 = idx_flat.reshape(N_CORES, T, 8, 16).transpose(0, 3, 1, 2).reshape(
        N_CORES, 16, T * 8)
    idx_arr = np.tile(w16, (1, 8, 1))               # [N_CORES, 128, T*8]

    # per-core epilogue scale layouts
    dinv_pad = np.ones(N_CORES * NPC, np.float32)
    sqdeg_pad = np.ones(N_CORES * NPC, np.float32)
    dinv_pad[:N_NODES] = dinv
    sqdeg_pad[:N_NODES] = sqdeg
    dinv_own = dinv_pad.reshape(N_CORES, NW, WSZ).transpose(0, 2, 1).copy()
    sqdeg_own = sqdeg_pad.reshape(N_CORES, 1, NPC).astype(ml_dtypes.bfloat16)

    sched = dict(T=T, runs=runs, tile_win=tile_win, n_th=n_th,
                 wfirst=wfirst, wlast=wlast, max_run=max_run)
    return idx_arr, vm8, dinv_own, sqdeg_own, sched


def _build_program(sched):
    import os
    VAR = set(os.environ.get("KVAR", "").split(","))
    import concourse.bass as bass
    import concourse.bacc as bacc
    import concourse.tile as tile
    from concourse import mybir

    f32 = mybir.dt.float32
    bf16 = mybir.dt.bfloat16
    fp8 = mybir.dt.float8e4
    i16 = mybir.dt.int16

    T = sched["T"]
    runs = sched["runs"]
    tile_win = sched["tile_win"]
    n_th = sched["n_th"]
    wfirst = sched["wfirst"]
    wlast = sched["wlast"]
    max_run = sched["max_run"]

    nsq = 4
    nc = bacc.Bacc("TRN2", target_bir_lowering=False, debug=False,
                   num_devices=N_CORES, num_swdge_queues=nsq)

    xT_own = nc.dram_tensor("xT_own", [64, NPC], bf16, kind="ExternalInput")
    src_idx = nc.dram_tensor("src_idx", [128, T * 8], i16, kind="ExternalInput")
    vm_in = nc.dram_tensor("vm8", [128, T * 128], fp8, kind="ExternalInput")
    dinv_in = nc.dram_tensor("dinv_own", [128, NW], f32, kind="ExternalInput")
    sqdeg_in = nc.dram_tensor("sqdeg_own", [1, NPC], bf16, kind="ExternalInput")
    Wmat = nc.dram_tensor("Wmat", [3, 64, 64], bf16, kind="ExternalInput")
    bias_in = nc.dram_tensor("bias_r", [3, 1, 64], bf16, kind="ExternalInput")
    ident_in = nc.dram_tensor("ident", [128, 128], bf16, kind="ExternalInput")
    agents = nc.dram_tensor("agents_out", [3, NW * 32, 64], bf16,
                            kind="ExternalOutput")

    with tile.TileContext(nc) as tc:
        with (
            tc.tile_pool(name="const", bufs=1) as constp,
            tc.tile_pool(name="hT", bufs=2) as hTp,
            tc.tile_pool(name="xws", bufs=2) as xwsp,
            tc.tile_pool(name="acc", bufs=2) as accp,
            tc.tile_pool(name="msg", bufs=6) as msgp,
            tc.tile_pool(name="vm", bufs=6) as vmp,
            tc.tile_pool(name="small", bufs=4) as smallp,
            tc.tile_pool(name="ps_seg", bufs=3, space="PSUM") as ps_seg,
            tc.tile_pool(name="ps_xw", bufs=2, space="PSUM") as ps_xw,
            tc.tile_pool(name="ps_tr", bufs=2, space="PSUM") as ps_tr,
            tc.tile_pool(name="dram_ag", bufs=1, space="DRAM") as dram_ag,
            tc.tile_pool(name="dram_xw", bufs=1, space="DRAM") as dram_xw,
        ):
            meta_idx = constp.tile([128, T * 8], i16)
            nc.sync.dma_start(out=meta_idx[:], in_=src_idx[:, :])
            dinv_t = constp.tile([128, NW], f32)
            sqdeg_t = constp.tile([1, NPC], bf16)
            ident_t = constp.tile([128, 128], bf16)
            nc.sync.dma_start(out=dinv_t[:], in_=dinv_in[:, :])
            nc.sync.dma_start(out=sqdeg_t[:], in_=sqdeg_in[:, :])
            nc.sync.dma_start(out=ident_t[:], in_=ident_in[:, :])
            w_tiles = []
            b_tiles = []
            for l in range(3):
                wt = constp.tile([64, 64], bf16, name=f"w{l}")
                bt = constp.tile([1, 64], bf16, name=f"b{l}")
                nc.sync.dma_start(out=wt[:], in_=Wmat[l, :, :])
                nc.sync.dma_start(out=bt[:], in_=bias_in[l, :, :])
                w_tiles.append(wt)
                b_tiles.append(bt)

            hT_cur = hTp.tile([64, NPC], bf16, tag="hT", name="hT0")
            nc.sync.dma_start(out=hT_cur[:], in_=xT_own[:, :])

            gg = 0      # global gather counter: Tile assigns SWDGE sems as
                        # gather#%8, so queue gather#%4 keeps sem<->queue 1:1
            NWLO = HALF_A // WSZ        # 25 windows feed table A
            hTs = {0: hT_cur}
            xws, ags, xwAs, xwBs = {}, {}, {}, {}

            def emit_xw_half(l, lo):
                """xw = (h @ W_l) * dinv for windows [0,25) or [25,49), plus
                the matching shard store + AllGather. The lo half of layer
                l+1 is emitted inside layer l so AG_a overlaps its PE tail."""
                if l not in xws:
                    xws[l] = xwsp.tile([128, NW * 64], bf16, tag="xws",
                                       name=f"xws{l}")
                    ags[l] = dram_ag.tile([NPC, 128], bf16, tag="ag",
                                          name=f"ag{l}")
                    xwAs[l] = dram_xw.tile([ROWS_A, 128], bf16, tag="xwA",
                                           addr_space="Shared",
                                           name=f"xwA{l}")
                    xwBs[l] = dram_xw.tile([ROWS_B, 128], bf16, tag="xwB",
                                           addr_space="Shared",
                                           name=f"xwB{l}")
                ws = range(0, NWLO) if lo else range(NWLO, NW)
                for w in ws:
                    ps = ps_xw.tile([128, 64], f32, tag="psxw",
                                    name=f"psxw{l}_{w}")
                    nc.tensor.matmul(
                        out=ps[:],
                        lhsT=hTs[l][:, w * 128:(w + 1) * 128],
                        rhs=w_tiles[l][:],
                        start=True, stop=True,
                    )
                    nc.scalar.mul(out=xws[l][:, w * 64:(w + 1) * 64],
                                  in_=ps[:], mul=dinv_t[:, w:w + 1])
                r0, r1 = (0, HALF_A) if lo else (HALF_A, NPC)
                nc.sync.dma_start(
                    out=ags[l][r0:r1, :].rearrange(
                        "(w p) f -> p w f", p=128)[:, :, 0:64],
                    in_=xws[l][:, r0 // 2:r1 // 2].rearrange(
                        "p (w f) -> p w f", f=64),
                )
                dst = xwAs[l] if lo else xwBs[l]
                if "nocoll" in VAR:
                    nc.sync.dma_start(out=dst[0:r1 - r0, :],
                                      in_=ags[l][r0:r1, :])
                else:
                    nc.gpsimd.collective_compute(
                        "AllGather",
                        mybir.AluOpType.bypass,
                        replica_groups=[list(range(N_CORES))],
                        ins=[ags[l][r0:r1, :].opt()],
                        outs=[dst.opt()],
                    )

            for l in range(3):
                if l == 0 or "nohoist" in VAR:
                    emit_xw_half(l, lo=True)
                xw_stage = xws[l]
                xwA, xwB = xwAs[l], xwBs[l]

                if l < 2:
                    hT_next = hTp.tile([64, NPC], bf16, tag="hT",
                                       name=f"hT{l + 1}")
                    hTs[l + 1] = hT_next
                else:
                    hT_next = None

                # parked phase-A partial sums, one [128, 64] slice per window
                acc_t = accp.tile([128, NW * 64], bf16, tag="acc",
                                  name=f"acc{l}")

                def epilogue(w, cur_ps):
                    hwin = smallp.tile([128, 64], bf16, tag="hwin",
                                       name=f"hw{l}_{w}")
                    nc.scalar.activation(
                        out=hwin[:], in_=cur_ps[:],
                        func=mybir.ActivationFunctionType.Sigmoid,
                        scale=dinv_t[:, w:w + 1],
                    )
                    nc.sync.dma_start(
                        out=agents[l, w * 32:(w + 1) * 32, :],
                        in_=hwin[0:128:4, :],
                    )
                    if hT_next is not None:
                        pt = ps_tr.tile([64, 128], bf16, tag="tr",
                                        name=f"tr{l}_{w}")
                        nc.tensor.transpose(out=pt[:], in_=hwin[:],
                                            identity=ident_t[:])
                        nc.scalar.copy(
                            out=hT_next[:, w * 128:(w + 1) * 128],
                            in_=pt[:],
                        )

                # ---- gather + two-phase windowed segment-sum ----
                # xw-hi + AG_b are emitted between the phase-A and phase-B
                # runs so the collective (in-order on gpsimd) cannot block
                # phase-A sub-gathers; windows >= NWLO therefore take their
                # self-loop term at phase-B open instead of phase-A open.
                # Only safe when every such window has phase-B tiles.
                late_hi = (all(n_th[1, w] > 0 for w in range(NWLO, NW))
                           and "latehi" not in VAR)
                if not late_hi:
                    emit_xw_half(l, lo=False)
                win_ps = {}
                hi_done = False
                for r, (t0, nt, h) in enumerate(runs):
                    if h == 1 and late_hi and not hi_done:
                        emit_xw_half(l, lo=False)
                        hi_done = True
                    msg = msgp.tile([128, max_run, 128], bf16, tag="msg",
                                    name=f"msg{l}_{r}")
                    vm_t = vmp.tile([128, max_run * 128], fp8, tag="vm",
                                    name=f"vm{l}_{r}")
                    nc.sync.dma_start(
                        out=vm_t[:, :nt * 128],
                        in_=vm_in[:, t0 * 128:(t0 + nt) * 128])
                    if "nogather" not in VAR:
                        # single_packet amortizes SDMA per-packet overhead but
                        # caps a call at 64 descs/engine = 8 tiles; sub-gathers
                        # share the run's queue so buffer/queue sems align
                        sp = "nosp" not in VAR
                        step = 8 if sp else nt
                        for s0 in range(0, nt, step):
                            sn = min(step, nt - s0)
                            nc.gpsimd.dma_gather(
                                out_ap=msg[:, s0:s0 + sn, :],
                                in_ap=(xwA[:] if h == 0 else xwB[:]),
                                idxs_ap=meta_idx[:, (t0 + s0) * 8:
                                                 (t0 + s0 + sn) * 8],
                                num_idxs=sn * 128,
                                num_idxs_reg=sn * 128,
                                elem_size=128,
                                single_packet=sp,
                                queue_num=(gg % nsq),
                            )
                            gg += 1
                    for j in range(nt):
                        t = t0 + j
                        w = int(tile_win[t])
                        if t == wfirst[h, w]:
                            cur = ps_seg.tile([128, 64], f32, tag="seg",
                                              name=f"seg{l}_{h}_{w}")
                            win_ps[w] = cur
                            # self-loop xw[w] is emitted before this point
                            # except for w >= NWLO under late_hi (phase B)
                            loop_here = (h == 1) if (late_hi and w >= NWLO) \
                                else (h == 0 or wfirst[0, w] < 0)
                            if h == 0 or wfirst[0, w] < 0:
                                # open with bias: psum = sqrt(deg) x bias
                                nc.tensor.matmul(
                                    out=cur[:],
                                    lhsT=sqdeg_t[:, w * 128:(w + 1) * 128],
                                    rhs=b_tiles[l][:],
                                    start=True, stop=False,
                                )
                            else:
                                # re-inject parked phase-A sum
                                nc.tensor.matmul(
                                    out=cur[:],
                                    lhsT=ident_t[:],
                                    rhs=acc_t[:, w * 64:(w + 1) * 64],
                                    start=True, stop=False,
                                )
                            if loop_here:
                                # self-loop term: += dinv[i] * xw[i]
                                nc.tensor.matmul(
                                    out=cur[:],
                                    lhsT=ident_t[:],
                                    rhs=xw_stage[:, w * 64:(w + 1) * 64],
                                    start=False, stop=False,
                                )
                        cur_ps = win_ps[w]
                        last = (t == wlast[h, w])
                        if "nomm" not in VAR:
                            nc.tensor.matmul(
                                out=cur_ps[:],
                                lhsT=vm_t[:, j * 128:(j + 1) * 128],
                                rhs=msg[:, j, 0:64],
                                start=False, stop=last,
                            )
                        elif last:
                            nc.scalar.copy(out=cur_ps[:], in_=cur_ps[:])
                        if last:
                            if h == 0 and wlast[1, w] >= 0:
                                # park phase-A sum in SBUF
                                nc.scalar.copy(
                                    out=acc_t[:, w * 64:(w + 1) * 64],
                                    in_=cur_ps[:])
                            else:
                                epilogue(w, cur_ps)

                # windows with no tiles at all (pad safety)
                for w in range(NW):
                    if wlast[0, w] < 0 and wlast[1, w] < 0:
                        cur = ps_seg.tile([128, 64], f32, tag="seg",
                                          name=f"segz{l}_{w}")
                        nc.tensor.matmul(
                            out=cur[:],
                            lhsT=sqdeg_t[:, w * 128:(w + 1) * 128],
                            rhs=b_tiles[l][:],
                            start=True, stop=False,
                        )
                        nc.tensor.matmul(
                            out=cur[:],
                            lhsT=ident_t[:],
                            rhs=xw_stage[:, w * 64:(w + 1) * 64],
                            start=False, stop=True,
                        )
                        epilogue(w, cur)

                if l < 2 and "nohoist" not in VAR:
                    # hoist next layer's lo-half xw + AG_a into this layer's
                    # PE/epilogue tail (windows 0-24's h are already final)
                    emit_xw_half(l + 1, lo=True)

                hT_cur = hT_next

    nc.compile()
    return nc


def kernel(**inputs):
    from concourse import bass_utils

    x = np.asarray(inputs["x"], dtype=np.float32)
    edge_index = np.asarray(inputs["edge_index"])
    agent_idx = np.asarray(inputs["agent_idx"], dtype=np.int64)
    Ws = [np.asarray(inputs[f"W{i}"], dtype=np.float32) for i in range(3)]
    bs = [np.asarray(inputs[f"b{i}"], dtype=np.float32) for i in range(3)]

    idx_arr, vm8, dinv_own, sqdeg_own, sched = _preprocess(edge_index)

    nc = _build_program(sched)

    xpad = np.zeros((N_CORES * NPC, D), np.float32)
    xpad[:N_NODES] = x
    Wstack = np.ascontiguousarray(
        np.stack(Ws)).astype(ml_dtypes.bfloat16)
    bias_stack = np.ascontiguousarray(
        np.stack([b[None, :] for b in bs])).astype(ml_dtypes.bfloat16)
    ident = np.eye(128, dtype=ml_dtypes.bfloat16)

    in_maps = []
    for c in range(N_CORES):
        in_maps.append({
            "xT_own": np.ascontiguousarray(
                xpad[c * NPC:(c + 1) * NPC].T).astype(ml_dtypes.bfloat16),
            "src_idx": np.ascontiguousarray(idx_arr[c]),
            "vm8": np.ascontiguousarray(vm8[c]),
            "dinv_own": np.ascontiguousarray(dinv_own[c]),
            "sqdeg_own": np.ascontiguousarray(sqdeg_own[c]),
            "Wmat": Wstack,
            "bias_r": bias_stack,
            "ident": ident,
        })

    res = bass_utils.run_bass_kernel_spmd(
        nc, in_maps, core_ids=list(range(N_CORES)))

    taps = np.stack([np.asarray(res.results[c]["agents_out"])
                     .astype(np.float32) for c in range(N_CORES)])
    # taps[c, l, r, :] = h_l for node (c*NPC + 4*r)
    n_agents = agent_idx.shape[0]
    out = np.empty((n_agents, 3 * D), np.float32)
    c_of = agent_idx // NPC
    r_of = (agent_idx % NPC) // 4
    for l in range(3):
        out[:, l * D:(l + 1) * D] = taps[c_of, l, r_of, :]
    return out
